# revision 68
# baseline (speedup 1.0000x reference)
"""Trainium2 Bass kernel for efficient-attention (nn_Attention_65532611003000).

Sharding: data-parallel over batch. B == n_cores == 8, so core i processes
batch element i end-to-end; no collectives are needed.

v3: fp8 (TRN e4m3) matmuls in DoubleRow / DoubleRowSwInterleave perf mode
for all the heavy GEMM streams (~2x PE columns/cycle); host-side precompute
of x+y (a third fp8 input) removes all device adds; DMA triggers batched
(2-chunk x transfers, per-group y transfers, paired output transfers);
pass-2 elementwise ops run at s-pair granularity to halve instruction +
semaphore overhead.

SwInterleave stationaries (kpre/vpre/qpre/sacc) are packed on the host in
the hardware's interleaved-and-column-reversed weight-stream order, which
roughly halves LDWEIGHTS time vs plain DoubleRow (the weight port is the
pass-1 bottleneck). For kpre's sacc consumer and for qpre, the column
reversal is pre-baked into the host layout (Wk's kch columns / Wq's
interleave) so the hardware reversal restores natural order downstream.

Scaling / folding tricks:
  - Wk/Wv/Wq are x16 on the host (raw ~U(+-0.044) entries are e4m3
    subnormal); exps apply scale=1/16. Wv's x16 is not undone: it rides
    v -> ctx -> cw and the host divides the output once at the end.
  - query bias bq is folded out of the exp: the zb stationary holds
    e^{bq}/64 (so Z is bias-correct), and cw rows carry e^{bq} (via the
    zkinv epilogue fold) incl. the br term, so out = sum qn'*cw is exact.
  - queries normalize with 64/Z so fp8 qn sits near 1.0; net output is
    x1024, divided back on the host; DRAM round-trip of out is bf16.

Per-core layout ([Nt, Ch] = [4096, 512], H=8 heads, 64 ch/head):
  pass 1 (per 128-token chunk i; DoubleRow pairs two 128-ch k-tiles):
    vpre = sum_tp DRS(xT_tp, Wv16_tp)      # [tok,512] psum
    kpre = sum_tp DRS(xsT_tp, Wk16rev_tp)  # xs = x+y packed on host
    khat = exp(kpre/16)                    # Act -> fp8, SwInterleave layout
    vaug = [vpre | ones]                   # DVE copy -> fp8, chunk pairs
    S_t += DRS(khat8_t, vaug_t) per pair   # ones cols accumulate Zk
  epilogue:
    ctx  = (S * zkinv * e^{bq}) per head   # blockdiag, 16x true ctx
  pass 2 (per group g of 4 chunks = 512 tokens; channel-major):
    per s-pair sp:
      qpre[u] = sum_tp DRS(Wq16_b[tp,2sp+u], yTg_tp)  # 2-bank psum pair
      qhatT[2sp:2sp+2] = exp(qpre_pair/16)            # one Act op per pair
      zb[u]   = (e^{bq} obd/64)^T @ qhatT[2sp+u]      # bf16, into pair psum
      zbinv   = recip_approx(zb_pair)                 # one DVE op per pair
      qn[sp]  = qhatT_pair * zbinv_pair               # DVE -> fp8
    g==0 overlap: CW[k] = e^{bq}[k]*(ctx_k^T Wr + 2 br_eff)  # fp8
    opre[j] = sum_sp DR(qn[sp][:,j], CW_sp)           # one group late
    osb copies on Act -> bf16 -> paired DMA out
"""

import sys

sys.path.insert(0, "/opt/trn_rl_repo")

import numpy as np
import ml_dtypes
from contextlib import ExitStack

import concourse.bass as bass
import concourse.bacc as bacc
import concourse.mybir as mybir
import concourse.tile as tile
from concourse.bass_utils import run_bass_kernel_spmd

B, Nt, Ch = 8, 4096, 512
H, HK = 8, 64
P = 128            # token chunk rows / SBUF partitions
NT = Nt // P       # 32 token chunks
CT = Ch // P       # 4 contraction tiles (2 DoubleRow pairs)
TP = CT // 2       # 2 k-tile pairs
GRP = 4            # pass-2 chunks per group (512 tokens)
NG = NT // GRP

W_SCALE = 16.0     # host scale on Wk/Wv/Wq
QN_SCALE = 64.0    # obd holds e^{bq}/QN_SCALE
OUT_SCALE = W_SCALE * QN_SCALE

F32 = mybir.dt.float32
BF16 = mybir.dt.bfloat16
FP8 = mybir.dt.float8e4
AX = mybir.AxisListType
AF = mybir.ActivationFunctionType
DR = mybir.MatmulPerfMode.DoubleRow
DRS = mybir.MatmulPerfMode.DoubleRowSwInterleave

BF16_NP = ml_dtypes.bfloat16
FP8_NP = ml_dtypes.float8_e4m3


def build_nc():
    nc = bacc.Bacc(None)

    # x and xs=(x+y) interleaved per chunk: [p, i, c, tp, u, q]
    xx_d = nc.declare_dram_parameter(
        "xxsT", [P, NT * 2 * CT * P], FP8, isOutput=False
    )
    yTg_d = nc.declare_dram_parameter(
        "yTg", [P, NG * CT * GRP * P], FP8, isOutput=False
    )
    wk_d = nc.declare_dram_parameter("Wk_r", [P, CT * Ch], FP8, isOutput=False)
    wv_d = nc.declare_dram_parameter("Wv_r", [P, CT * Ch], FP8, isOutput=False)
    wr_d = nc.declare_dram_parameter("Wr_r", [P, CT * Ch], BF16, isOutput=False)
    wqb_d = nc.declare_dram_parameter("Wq_b", [P, CT * CT * P], FP8, isOutput=False)
    brb_d = nc.declare_dram_parameter("brb_t", [P, CT * Ch], F32, isOutput=False)
    ebq_d = nc.declare_dram_parameter("ebq_col", [P, CT], F32, isOutput=False)
    bvb_d = nc.declare_dram_parameter("bv_blk", [P, Ch], BF16, isOutput=False)
    obd_d = nc.declare_dram_parameter("onesbd", [P, CT * P], BF16, isOutput=False)
    id16_d = nc.declare_dram_parameter("ident16", [P, P], BF16, isOutput=False)
    onescol_d = nc.declare_dram_parameter(
        "ones_col", [P, 2 * CT * 8], FP8, isOutput=False
    )
    out_d = nc.declare_dram_parameter("out", [Nt, Ch], BF16, isOutput=True)

    with tile.TileContext(nc) as tc, ExitStack() as ctx:
        const = ctx.enter_context(tc.tile_pool(name="const", bufs=1))

        wk = const.tile([P, TP, 2, Ch], FP8)
        wv = const.tile([P, TP, 2, Ch], FP8)
        wr = const.tile([P, CT, Ch], BF16)
        wqb = const.tile([P, TP, CT, P, 2], FP8)   # SwInterleave layout
        brb = const.tile([P, CT, Ch], F32)
        ebq = const.tile([P, CT], F32)
        bvb = const.tile([P, Ch], BF16)
        obd = const.tile([P, CT, P], BF16)
        id16 = const.tile([P, P], BF16)
        cw = const.tile([P, TP, 2, Ch], FP8)           # ctx @ Wr, built once
        # resident x,xs; last dims (j, b): column-reversed A/B-interleaved
        # physical layout for DoubleRowSwInterleave stationaries, 32KB/part
        xx_all = const.tile([P, NT, 2, TP, P, 2], FP8)
        yTg_all = const.tile([P, NG, TP, 2, GRP * P], FP8)
        ctxR = const.tile([P, CT, P], BF16)            # per-head ctx, blockdiag
        zkinv = const.tile([P, CT], F32)

        xx_v = xx_d[:].rearrange(
            "p (i c a q b) -> p i c a q b", i=NT, c=2, a=TP, b=2
        )
        # startup triggers split across the two hardware-DGE engines (SP and
        # Act) so the serial ~650ns-per-trigger chain is halved
        wv_v = wv_d[:].rearrange("p (a b j) -> p a b j", a=TP, b=2)
        wk_v = wk_d[:].rearrange("p (a b j) -> p a b j", a=TP, b=2)
        nc.sync.dma_start(xx_all[:, 0:1], xx_v[:, 0:1])
        nc.scalar.dma_start(wv[:, 0:1], wv_v[:, 0:1])
        nc.sync.dma_start(wv[:, 1:2], wv_v[:, 1:2])
        nc.scalar.dma_start(wk[:, 0:1], wk_v[:, 0:1])
        nc.sync.dma_start(wk[:, 1:2], wk_v[:, 1:2])
        nc.scalar.dma_start(xx_all[:, 1:2], xx_v[:, 1:2])
        nc.sync.dma_start(xx_all[:, 2:4], xx_v[:, 2:4])
        nc.scalar.dma_start(xx_all[:, 4:6], xx_v[:, 4:6])

        yTg_v = yTg_d[:].rearrange(
            "p (g a b j) -> p g a b j", g=NG, a=TP, b=2
        )

        # ---------------- pass 1: khat, v, S & Zk accumulation --------------
        VW = 136   # vaug cols padded so the chunk-pair stride is 16B-aligned
        NPAIR = NT // 2
        # group 0's query exps, precomputed during pass 1's tail so pass 2's
        # DVE softmax chain starts immediately at the pass boundary
        qhatT0 = const.tile([P, CT, GRP * P], BF16)
        with (
            tc.tile_pool(name="sb1", bufs=2) as sb1,
            tc.tile_pool(name="ps_k", bufs=2, space="PSUM") as ps_k,
            tc.tile_pool(name="ps_v", bufs=2, space="PSUM") as ps_v,
            tc.tile_pool(name="ps_s", bufs=1, space="PSUM") as ps_s,
        ):
            # NOTE: each s_acc accumulator needs its OWN psum bank — a
            # matmul with start=True clears has_written for the whole bank,
            # so two independent accumulation groups must not share one.
            s_tiles = [
                ps_s.tile([P, VW], F32, tag=f"sacc{t}", name=f"sacc{t}")
                for t in range(CT)
            ]
            # chunk-PAIR buffers for the fp8 DoubleRow sacc: dim1/b = chunk
            # parity (the two k-tiles of the token contraction). khat8 holds
            # Wk-column-reversed interleaved stationaries (host reverses Wk's
            # kch columns, so plain positive-stride exp writes produce the
            # SwInterleave layout and the HW reversal restores kch order).
            v_aug_bufs = [
                sb1.tile([P, 2, CT, VW], FP8, tag=f"vaug{n}", name=f"vaug{n}")
                for n in range(2)
            ]
            khat_bufs = [
                sb1.tile([P, CT, P, 2], FP8, tag=f"khat{n}", name=f"khat{n}")
                for n in range(2)
            ]
            for n in range(2):
                nc.sync.dma_start(
                    v_aug_bufs[n][:, :, :, 128:VW],
                    onescol_d[:].rearrange(
                        "p (b t c) -> p b t c", b=2, t=CT
                    ),
                )

            # HAM warmup: the PE otherwise idles ~4us behind the startup DMA
            # chain and then runs its first ~3.4us at the cold 1.2 GHz clock.
            # Busy-spin it on a const AP (memset during the preamble, so no
            # DMA dependency) so the clock gate is already at 8/8 when the
            # real matmuls arrive.
            warm_ps = ps_k.tile([P, Ch], F32, tag="kpre", name="warm")
            warm_c = nc.const_aps.aps[(mybir.dt.bfloat16, 1.0)]
            # moving operand is an uninitialized scratch tile: its values are
            # garbage but the output is never read; N=128 makes each spin
            # ~107ns so ~33 of them cover the 3.4us HAM activity window
            warm_src = sb1.tile([P, P], BF16, tag="warmsrc", name="warmsrc")
            nc.gpsimd.memset(warm_src[:], 0)
            for _ in range(34):
                nc.tensor.matmul(
                    warm_ps[0:1, 0:P],
                    warm_c,
                    warm_src[:],
                    start=True,
                    stop=True,
                )

            def emit_sacc(pr):
                kh = khat_bufs[pr % 2]
                va = v_aug_bufs[pr % 2]
                for t in range(CT):
                    nc.tensor.matmul(
                        s_tiles[t][:],
                        kh[:, t, :, :],
                        va[:, :, t, :],
                        start=(pr == 0),
                        stop=(pr == NPAIR - 1),
                        perf_mode=DRS,
                    )

            for i in range(NT):
                # paired input DMAs, issued six chunks ahead (pairs 0-2 were
                # issued before the loop)
                if i % 2 == 0 and i + 6 < NT:
                    nc.sync.dma_start(
                        xx_all[:, i + 6 : i + 8], xx_v[:, i + 6 : i + 8]
                    )
                # pass-2/epilogue consts: spread one per chunk so they never
                # clump the SP trigger queue against the xx prefetches
                if i == 3:
                    nc.sync.dma_start(
                        wqb[:],
                        wqb_d[:].rearrange(
                            "p (a s j b) -> p a s j b", a=TP, s=CT, b=2
                        ),
                    )
                elif i == 5:
                    nc.sync.dma_start(
                        wr[:], wr_d[:].rearrange("p (t j) -> p t j", t=CT)
                    )
                elif i == 7:
                    nc.sync.dma_start(
                        brb[:], brb_d[:].rearrange("p (t j) -> p t j", t=CT)
                    )
                elif i == 9:
                    nc.sync.dma_start(ebq[:], ebq_d[:])
                    nc.sync.dma_start(bvb[:], bvb_d[:])
                elif i == 11:
                    nc.sync.dma_start(
                        obd[:], obd_d[:].rearrange("p (s q) -> p s q", s=CT)
                    )
                elif i == 13:
                    nc.sync.dma_start(id16[:], id16_d[:])
                elif i >= 17 and i % 2 == 1:
                    g_pre = (i - 17) // 2   # prefetch y for pass 2
                    nc.sync.dma_start(yTg_all[:, g_pre], yTg_v[:, g_pre])

                vpre = ps_v.tile([P, Ch], F32, tag="vpre")
                for tp in range(TP):
                    nc.tensor.matmul(
                        vpre[:],
                        xx_all[:, i, 0, tp, :, :],
                        wv[:, tp, :, :],
                        start=(tp == 0),
                        stop=(tp == TP - 1),
                        perf_mode=DRS,
                    )
                # previous pair's S accumulation, delayed one chunk so its
                # khat LDWEIGHTS pulls ahead under the vpre stream instead
                # of chasing the Act exp
                if i >= 2 and i % 2 == 0:
                    emit_sacc(i // 2 - 1)
                v_aug = v_aug_bufs[(i // 2) % 2]
                nc.vector.tensor_copy(
                    v_aug[:, i % 2, :, 0:128],
                    vpre[:].rearrange("p (t q) -> p t q", t=CT),
                )

                kpre = ps_k.tile([P, Ch], F32, tag="kpre")
                for tp in range(TP):
                    nc.tensor.matmul(
                        kpre[:],
                        xx_all[:, i, 1, tp, :, :],
                        wk[:, tp, :, :],
                        start=(tp == 0),
                        stop=(tp == TP - 1),
                        perf_mode=DRS,
                    )
                if i >= NT - 4:
                    # group-0 qpre + exp (one s per chunk), overlapped with
                    # the pass-1 tail; psum comes from the vpre ring slot
                    # that the ring would hand out next anyway
                    s = i - (NT - 4)
                    qpre0 = ps_v.tile([P, GRP * P], F32, tag="vpre")
                    for tp in range(TP):
                        nc.tensor.matmul(
                            qpre0[:],
                            wqb[:, tp, s, :, :],
                            yTg_all[:, 0, tp, :, :],
                            start=(tp == 0),
                            stop=(tp == TP - 1),
                            perf_mode=DRS,
                        )
                    nc.scalar.activation(
                        qhatT0[:, s, :],
                        qpre0[:],
                        AF.Exp,
                        scale=1.0 / W_SCALE,
                    )
                khat8 = khat_bufs[(i // 2) % 2]
                if i == NT - 1:
                    # split the final exp per block so the last s_acc
                    # matmuls (and the chain behind them) start a
                    # quarter-exp earlier instead of idling the PE
                    for t in range(CT):
                        nc.scalar.activation(
                            khat8[:, t, :, i % 2],
                            kpre[:, P * t : P * (t + 1)],
                            AF.Exp,
                            scale=1.0 / W_SCALE,
                        )
                else:
                    nc.scalar.activation(
                        khat8[:, :, :, i % 2],
                        kpre[:].rearrange("p (t q) -> p t q", t=CT),
                        AF.Exp,
                        scale=1.0 / W_SCALE,
                    )
            emit_sacc(NPAIR - 1)

            # ------- epilogue: ctx = S * zkinv * e^{bq}  (x16 scale) --------
            for t in range(CT):
                nc.vector.reciprocal(
                    zkinv[:, t : t + 1], s_tiles[t][:, 128:129]
                )
            # fold the query-bias factor into the ctx rows (kch partitions)
            nc.vector.tensor_mul(zkinv[:], zkinv[:], ebq[:])
            for t in range(CT):
                nc.vector.tensor_copy(ctxR[:, t, :], bvb[:, P * t : P * (t + 1)])
                for blk in range(2):
                    p0 = 64 * blk
                    nc.vector.scalar_tensor_tensor(
                        ctxR[p0 : p0 + 64, t, p0 : p0 + 64],
                        s_tiles[t][p0 : p0 + 64, p0 : p0 + 64],
                        zkinv[p0 : p0 + 64, t : t + 1],
                        bvb[p0 : p0 + 64, P * t + p0 : P * t + p0 + 64],
                        op0=mybir.AluOpType.mult,
                        op1=mybir.AluOpType.add,
                    )

        # ---------------- pass 2: q softmax, attend, reproject ---------------
        # s-pair granularity; the reprojection runs one group late so the PE
        # never stalls waiting for the DVE/Pool multiply chain.
        with (
            tc.tile_pool(name="io2", bufs=2) as io2,
            tc.tile_pool(name="sb2", bufs=2) as sb2,
            tc.tile_pool(name="ps_q", bufs=2, space="PSUM") as ps_q,
            tc.tile_pool(name="ps_zb", bufs=1, space="PSUM") as ps_zb,
            tc.tile_pool(name="ps_o", bufs=2, space="PSUM") as ps_o,
        ):
            qn_bufs = [
                sb2.tile([P, TP, 2, GRP * P], FP8, tag=f"qn{n}", name=f"qn{n}")
                for n in range(2)
            ]

            def emit_opre_half(g, jp, dve_osb=False):
                qn = qn_bufs[g % 2]
                o_sb = io2.tile([P, 2, Ch], BF16, tag="osb")
                for jj in range(2):
                    j = 2 * jp + jj
                    opre = ps_o.tile([P, Ch], F32, tag="opre")
                    for sp in range(TP):
                        nc.tensor.matmul(
                            opre[:],
                            qn[:, sp, :, P * j : P * (j + 1)],
                            cw[:, sp, :, :],
                            start=(sp == 0),
                            stop=(sp == TP - 1),
                            perf_mode=DR,
                        )
                    # the tail groups run after all DVE softmax work is done,
                    # so splitting the copies Act/DVE halves the drain time
                    if dve_osb and jj == 1:
                        nc.vector.tensor_copy(o_sb[:, jj, :], opre[:])
                    else:
                        nc.scalar.copy(o_sb[:, jj, :], opre[:])
                i0 = g * GRP + 2 * jp
                nc.sync.dma_start(
                    out_d[P * i0 : P * (i0 + 2), :].rearrange(
                        "(u p) c -> p u c", u=2
                    ),
                    o_sb[:],
                )

            for g in range(NG):
                if g == 0:
                    qhatT = qhatT0
                else:
                    qhatT = sb2.tile([P, CT, GRP * P], BF16, tag="qhatT")
                zbinv = sb2.tile([P, CT, GRP * P], F32, tag="zbinv")
                qn = qn_bufs[g % 2]

                def emit_zb(sp, qhatT=qhatT, zbinv=zbinv, qn=qn):
                    zbp = ps_zb.tile([P, 2, GRP * P], F32, tag="zb")
                    for u in range(2):
                        nc.tensor.matmul(
                            zbp[:, u, :],
                            obd[:, 2 * sp + u, :],
                            qhatT[:, 2 * sp + u, :],
                            start=True,
                            stop=True,
                        )
                    nc.vector.reciprocal_approx_fast(
                        zbinv[:, 2 * sp : 2 * sp + 2, :], zbp[:]
                    )
                    # qn = qhat * (64/Z), one DVE op per s-pair
                    nc.vector.tensor_mul(
                        qn[:, sp, :, :],
                        qhatT[:, 2 * sp : 2 * sp + 2, :],
                        zbinv[:, 2 * sp : 2 * sp + 2, :],
                    )

                for sp in range(TP):
                    if g > 0:
                        qpre = ps_q.tile([P, 2, GRP * P], F32, tag="qpre")
                        for u in range(2):
                            s = 2 * sp + u
                            for tp in range(TP):
                                nc.tensor.matmul(
                                    qpre[:, u, :],
                                    wqb[:, tp, s, :, :],
                                    yTg_all[:, g, tp, :, :],
                                    start=(tp == 0),
                                    stop=(tp == TP - 1),
                                    perf_mode=DRS,
                                )
                        nc.scalar.activation(
                            qhatT[:, 2 * sp : 2 * sp + 2, :],
                            qpre[:],
                            AF.Exp,
                            scale=1.0 / W_SCALE,
                        )
                    # zb[p,tok] = (1/64)*sum_{k in head(p)} e^{bq[k]} qhat[k]:
                    # bias-corrected Z/64 pre-broadcast on all 128 partitions.
                    # Delayed one pair so it never chases the Act exp; feeds
                    # opre one group later, so the reciprocal never gates PE.
                    if sp > 0:
                        emit_zb(sp - 1)

                if g == 0:
                    # CW[k,c] = e^{bq[k]}*(sum_v ctx[k,v] Wr[v,c] + 2 br_eff):
                    # ctx already carries e^{bq} (zkinv fold), brb_t rows are
                    # host-scaled. sum_k qn'[k] e^{bq[k]} per head = 64, so
                    # out_raw = 1024*(attended@Wr + br_eff). Emitted here so
                    # the wait on the ctx epilogue overlaps group 0's qpres.
                    for t in range(CT):
                        ctxT_ps = ps_q.tile([P, P], BF16, tag="qpre")
                        nc.tensor.transpose(ctxT_ps[:], ctxR[:, t, :], id16[:])
                        ctxT = sb2.tile([P, P], BF16, tag="ctxTs")
                        nc.scalar.copy(ctxT[:], ctxT_ps[:])
                        cw_ps = ps_q.tile([P, Ch], F32, tag="qpre")
                        nc.tensor.matmul(
                            cw_ps[:], ctxT[:], wr[:, t, :], start=True, stop=True
                        )
                        nc.vector.tensor_add(
                            cw[:, t // 2, t % 2, :], cw_ps[:], brb[:, t, :]
                        )
                    emit_zb(TP - 1)
                else:
                    # interleave the last zb between the two reprojection
                    # halves so its moving qhat never chases the Act exp and
                    # the fused divide starts early enough for the next group
                    emit_opre_half(g - 1, 0)
                    emit_zb(TP - 1)
                    emit_opre_half(g - 1, 1)

            emit_opre_half(NG - 1, 0, dve_osb=True)
            emit_opre_half(NG - 1, 1, dve_osb=True)

    nc.finalize()
    return nc


def _host_consts(Wk, bk, Wq, bq, Wv, bv, Wr, br):
    def rearr8(w):
        return (
            np.ascontiguousarray(
                (w * W_SCALE)
                .reshape(CT, P, Ch)
                .transpose(1, 0, 2)
                .reshape(P, CT * Ch)
            ).astype(FP8_NP)
        )

    # Wq x16, SwInterleave stationary layout [p, tp, s, j, b] with the
    # column reversal baked in so the HW reversal restores kch order:
    # wqb[p, tp, s, j, b] = 16*Wq[(2tp+b)*128 + p, s*128 + (127-j)]
    wq = (Wq * W_SCALE).reshape(TP, 2, P, CT, P)   # [tp, b, p, s, q]
    wq = wq[..., ::-1]                             # q -> j
    wqb = np.ascontiguousarray(
        wq.transpose(2, 0, 3, 4, 1).reshape(P, CT * CT * P)
    ).astype(FP8_NP)

    wr_r = np.ascontiguousarray(
        Wr.reshape(CT, P, Ch).transpose(1, 0, 2).reshape(P, CT * Ch)
    ).astype(BF16_NP)

    # e^{bq} rounded to bf16 once so numerator (cw rows) and denominator
    # (obd) use the *same* values; /64 below is exact (exponent shift).
    ebq = np.exp(bq.astype(np.float64)).astype(BF16_NP).astype(np.float32)
    ebq_col = np.ascontiguousarray(ebq.reshape(CT, P).T).astype(np.float32)

    # ctx is built WITHOUT bv (bv @ Wr is absorbed into br below, exact
    # because the query softmax weights sum to 1); bvb is all zeros so the
    # epilogue's fused multiply-add writes S*zkinv*e^{bq} on the diagonal.
    bvb = np.zeros((P, Ch), np.float32)
    br_eff = (
        br.astype(np.float64) + bv.astype(np.float64) @ Wr.astype(np.float64)
    ).astype(np.float32)

    # brb_t[p, t, c] = e^{bq[t*128+p]} * (W_SCALE/8) * br_eff[c]
    brb_t = (
        ebq.reshape(CT, P).transpose(1, 0)[:, :, None]
        * (br_eff[None, None, :] * (W_SCALE / 8.0))
    ).reshape(P, CT * Ch)

    # per-s blockdiag stationaries: obd[p, s, m] = e^{bq[s*128+p]}/64 where
    # p, m are in the same head-half (the e^{bq} factor rides the contraction
    # row, i.e. the absolute kch index k = s*128 + p)
    mask = np.zeros((P, P), np.float32)
    mask[0:64, 0:64] = 1.0
    mask[64:128, 64:128] = 1.0
    ebq_sp = ebq.reshape(CT, P)  # [s, p]
    obd = (
        mask[:, None, :] * ebq_sp.transpose(1, 0)[:, :, None] / QN_SCALE
    ).reshape(P, CT * P)
    # Wk's kch output columns are reversed within each 128-tile: the exp then
    # writes khat in SwInterleave physical order with plain strides, and the
    # DoubleRowSwInterleave hardware reversal restores kch order in s_acc.
    Wk_rev = np.ascontiguousarray(
        Wk.reshape(Ch, CT, P)[:, :, ::-1].reshape(Ch, Ch)
    )
    return {
        "Wk_r": rearr8(Wk_rev),
        "Wv_r": rearr8(Wv),
        "Wr_r": wr_r,
        "Wq_b": wqb,
        "brb_t": np.ascontiguousarray(brb_t).astype(np.float32),
        "ebq_col": ebq_col,
        "bv_blk": bvb.astype(BF16_NP),
        "onesbd": np.ascontiguousarray(obd).astype(BF16_NP),
        "ident16": np.eye(P).astype(BF16_NP),
        "ones_col": np.ones((P, 2 * CT * 8), FP8_NP),
    }


def _pack_xx(x, y):
    """SwInterleave stationary layout:
    xxsT[p, i, c, tp, j, b] = {x, x+y}[i*128 + (127-j), (2*tp+b)*128 + p].
    (A/B pairs interleaved along columns, column order reversed, per the
    DoubleRowSwInterleave hardware weight-stream convention.)"""

    def lay(a):
        t = a.reshape(NT, P, TP, 2, P).transpose(4, 0, 2, 3, 1)  # p i tp b q
        t = t[..., ::-1]                                         # q -> j
        return t.transpose(0, 1, 2, 4, 3)                        # p i tp j b

    return (
        np.stack([lay(x), lay(x + y)], axis=2)
        .astype(FP8_NP)
        .reshape(P, NT * 2 * CT * P)
    )


def _pack_y(y):
    """yTg[p, g, tp, u, j] = y[g*512+j, (2*tp+u)*128+p]."""
    return np.ascontiguousarray(
        y.reshape(NG, GRP * P, TP, 2, P)
        .transpose(4, 0, 2, 3, 1)
        .reshape(P, NG * CT * GRP * P)
    ).astype(FP8_NP)


_NC_CACHE = {}


def _get_nc():
    if "nc" not in _NC_CACHE:
        _NC_CACHE["nc"] = build_nc()
    return _NC_CACHE["nc"]


def kernel(input_, y, Wk, bk, Wq, bq, Wv, bv, Wr, br, _trace=False, _tmpdir=None):
    input_ = np.asarray(input_, np.float32)
    y = np.asarray(y, np.float32)
    consts = _host_consts(
        np.asarray(Wk, np.float32), np.asarray(bk, np.float32),
        np.asarray(Wq, np.float32), np.asarray(bq, np.float32),
        np.asarray(Wv, np.float32), np.asarray(bv, np.float32),
        np.asarray(Wr, np.float32), np.asarray(br, np.float32),
    )
    nc = _get_nc()
    in_maps = [
        {
            "xxsT": _pack_xx(input_[i], y[i]),
            "yTg": _pack_y(y[i]),
            **consts,
        }
        for i in range(B)
    ]
    res = run_bass_kernel_spmd(
        nc, in_maps, core_ids=list(range(B)), trace=_trace, tmpdir=_tmpdir
    )
    out = np.stack(
        [
            res.results[i]["out"].astype(np.float32) * (1.0 / OUT_SCALE)
            for i in range(B)
        ],
        axis=0,
    )
    if _trace:
        return out, res
    return out


# revision 73
# speedup vs baseline: 1.0086x; 1.0086x over previous
"""Trainium2 Bass kernel for efficient-attention (nn_Attention_65532611003000).

Sharding: data-parallel over batch. B == n_cores == 8, so core i processes
batch element i end-to-end; no collectives are needed.

v3: fp8 (TRN e4m3) matmuls in DoubleRow / DoubleRowSwInterleave perf mode
for all the heavy GEMM streams (~2x PE columns/cycle); host-side precompute
of x+y (a third fp8 input) removes all device adds; DMA triggers batched
(2-chunk x transfers, per-group y transfers, paired output transfers);
pass-2 elementwise ops run at s-pair granularity to halve instruction +
semaphore overhead.

SwInterleave stationaries (kpre/vpre/qpre/sacc) are packed on the host in
the hardware's interleaved-and-column-reversed weight-stream order, which
roughly halves LDWEIGHTS time vs plain DoubleRow (the weight port is the
pass-1 bottleneck). For kpre's sacc consumer and for qpre, the column
reversal is pre-baked into the host layout (Wk's kch columns / Wq's
interleave) so the hardware reversal restores natural order downstream.

Scaling / folding tricks:
  - Wk/Wv/Wq are x16 on the host (raw ~U(+-0.044) entries are e4m3
    subnormal); exps apply scale=1/16. Wv's x16 is not undone: it rides
    v -> ctx -> cw and the host divides the output once at the end.
  - query bias bq is folded out of the exp: the zb stationary holds
    e^{bq}/64 (so Z is bias-correct), and cw rows carry e^{bq} (via the
    zkinv epilogue fold) incl. the br term, so out = sum qn'*cw is exact.
  - queries normalize with 64/Z so fp8 qn sits near 1.0; net output is
    x1024, divided back on the host; DRAM round-trip of out is bf16.

Per-core layout ([Nt, Ch] = [4096, 512], H=8 heads, 64 ch/head):
  pass 1 (per 128-token chunk i; DoubleRow pairs two 128-ch k-tiles):
    vpre = sum_tp DRS(xT_tp, Wv16_tp)      # [tok,512] psum
    kpre = sum_tp DRS(xsT_tp, Wk16rev_tp)  # xs = x+y packed on host
    khat = exp(kpre/16)                    # Act -> fp8, SwInterleave layout
    vaug = [vpre | ones]                   # DVE copy -> fp8, chunk pairs
    S_t += DRS(khat8_t, vaug_t) per pair   # ones cols accumulate Zk
  epilogue:
    ctx  = (S * zkinv * e^{bq}) per head   # blockdiag, 16x true ctx
  pass 2 (per group g of 4 chunks = 512 tokens; channel-major):
    per s-pair sp:
      qpre[u] = sum_tp DRS(Wq16_b[tp,2sp+u], yTg_tp)  # 2-bank psum pair
      qhatT[2sp:2sp+2] = exp(qpre_pair/16)            # one Act op per pair
      zb[u]   = (e^{bq} obd/64)^T @ qhatT[2sp+u]      # bf16, into pair psum
      zbinv   = recip_approx(zb_pair)                 # one DVE op per pair
      qn[sp]  = qhatT_pair * zbinv_pair               # DVE -> fp8
    g==0 overlap: CW[k] = e^{bq}[k]*(ctx_k^T Wr + 2 br_eff)  # fp8
    opre[j] = sum_sp DR(qn[sp][:,j], CW_sp)           # one group late
    osb copies on Act -> bf16 -> paired DMA out
"""

import sys

sys.path.insert(0, "/opt/trn_rl_repo")

import numpy as np
import ml_dtypes
from contextlib import ExitStack

import concourse.bass as bass
import concourse.bacc as bacc
import concourse.mybir as mybir
import concourse.tile as tile
from concourse.bass_utils import run_bass_kernel_spmd

B, Nt, Ch = 8, 4096, 512
H, HK = 8, 64
P = 128            # token chunk rows / SBUF partitions
NT = Nt // P       # 32 token chunks
CT = Ch // P       # 4 contraction tiles (2 DoubleRow pairs)
TP = CT // 2       # 2 k-tile pairs
GRP = 4            # pass-2 chunks per group (512 tokens)
NG = NT // GRP

W_SCALE = 16.0     # host scale on Wk/Wv/Wq
QN_SCALE = 64.0    # obd holds e^{bq}/QN_SCALE
OUT_SCALE = W_SCALE * QN_SCALE

F32 = mybir.dt.float32
BF16 = mybir.dt.bfloat16
FP8 = mybir.dt.float8e4
AX = mybir.AxisListType
AF = mybir.ActivationFunctionType
DR = mybir.MatmulPerfMode.DoubleRow
DRS = mybir.MatmulPerfMode.DoubleRowSwInterleave

BF16_NP = ml_dtypes.bfloat16
FP8_NP = ml_dtypes.float8_e4m3


def build_nc():
    nc = bacc.Bacc(None)

    # x and xs=(x+y) interleaved per chunk: [p, i, c, tp, u, q]
    xx_d = nc.declare_dram_parameter(
        "xxsT", [P, NT * 2 * CT * P], FP8, isOutput=False
    )
    yTg_d = nc.declare_dram_parameter(
        "yTg", [P, NG * CT * GRP * P], FP8, isOutput=False
    )
    wk_d = nc.declare_dram_parameter("Wk_r", [P, CT * Ch], FP8, isOutput=False)
    wv_d = nc.declare_dram_parameter("Wv_r", [P, CT * Ch], FP8, isOutput=False)
    wr_d = nc.declare_dram_parameter("Wr_r", [P, CT * Ch], BF16, isOutput=False)
    wqb_d = nc.declare_dram_parameter("Wq_b", [P, CT * CT * P], FP8, isOutput=False)
    brb_d = nc.declare_dram_parameter("brb_t", [P, CT * Ch], F32, isOutput=False)
    ebq_d = nc.declare_dram_parameter("ebq_col", [P, CT], F32, isOutput=False)
    bvb_d = nc.declare_dram_parameter("bv_blk", [P, Ch], BF16, isOutput=False)
    obd_d = nc.declare_dram_parameter("onesbd", [P, CT * P], BF16, isOutput=False)
    id16_d = nc.declare_dram_parameter("ident16", [P, P], BF16, isOutput=False)
    onescol_d = nc.declare_dram_parameter(
        "ones_col", [P, 2 * CT * 8], FP8, isOutput=False
    )
    out_d = nc.declare_dram_parameter("out", [Nt, Ch], BF16, isOutput=True)

    with tile.TileContext(nc) as tc, ExitStack() as ctx:
        const = ctx.enter_context(tc.tile_pool(name="const", bufs=1))

        wk = const.tile([P, TP, 2, Ch], FP8)
        wv = const.tile([P, TP, 2, Ch], FP8)
        wr = const.tile([P, CT, Ch], BF16)
        wqb = const.tile([P, TP, CT, P, 2], FP8)   # SwInterleave layout
        brb = const.tile([P, CT, Ch], F32)
        ebq = const.tile([P, CT], F32)
        bvb = const.tile([P, Ch], BF16)
        obd = const.tile([P, CT, P], BF16)
        id16 = const.tile([P, P], BF16)
        cw = const.tile([P, TP, 2, Ch], FP8)           # ctx @ Wr, built once
        # resident x,xs; last dims (j, b): column-reversed A/B-interleaved
        # physical layout for DoubleRowSwInterleave stationaries, 32KB/part
        xx_all = const.tile([P, NT, 2, TP, P, 2], FP8)
        yTg_all = const.tile([P, NG, TP, 2, GRP * P], FP8)
        ctxR = const.tile([P, CT, P], BF16)            # per-head ctx, blockdiag
        zkinv = const.tile([P, CT], F32)

        xx_v = xx_d[:].rearrange(
            "p (i c a q b) -> p i c a q b", i=NT, c=2, a=TP, b=2
        )
        # startup triggers split across the two hardware-DGE engines (SP and
        # Act) so the serial ~650ns-per-trigger chain is halved
        wv_v = wv_d[:].rearrange("p (a b j) -> p a b j", a=TP, b=2)
        wk_v = wk_d[:].rearrange("p (a b j) -> p a b j", a=TP, b=2)
        nc.sync.dma_start(xx_all[:, 0:1], xx_v[:, 0:1])
        nc.scalar.dma_start(wv[:, 0:1], wv_v[:, 0:1])
        nc.sync.dma_start(wv[:, 1:2], wv_v[:, 1:2])
        nc.scalar.dma_start(wk[:, 0:1], wk_v[:, 0:1])
        nc.sync.dma_start(wk[:, 1:2], wk_v[:, 1:2])
        nc.scalar.dma_start(xx_all[:, 1:2], xx_v[:, 1:2])
        nc.sync.dma_start(xx_all[:, 2:4], xx_v[:, 2:4])
        nc.scalar.dma_start(xx_all[:, 4:6], xx_v[:, 4:6])

        yTg_v = yTg_d[:].rearrange(
            "p (g a b j) -> p g a b j", g=NG, a=TP, b=2
        )

        # ---------------- pass 1: khat, v, S & Zk accumulation --------------
        VW = 136   # vaug cols padded so the chunk-pair stride is 16B-aligned
        NPAIR = NT // 2
        with (
            tc.tile_pool(name="sb1", bufs=2) as sb1,
            tc.tile_pool(name="ps_k", bufs=2, space="PSUM") as ps_k,
            tc.tile_pool(name="ps_v", bufs=2, space="PSUM") as ps_v,
            tc.tile_pool(name="ps_s", bufs=1, space="PSUM") as ps_s,
        ):
            # NOTE: each s_acc accumulator needs its OWN psum bank — a
            # matmul with start=True clears has_written for the whole bank,
            # so two independent accumulation groups must not share one.
            s_tiles = [
                ps_s.tile([P, VW], F32, tag=f"sacc{t}", name=f"sacc{t}")
                for t in range(CT)
            ]
            # chunk-PAIR buffers for the fp8 DoubleRow sacc: dim1/b = chunk
            # parity (the two k-tiles of the token contraction). khat8 holds
            # Wk-column-reversed interleaved stationaries (host reverses Wk's
            # kch columns, so plain positive-stride exp writes produce the
            # SwInterleave layout and the HW reversal restores kch order).
            v_aug_bufs = [
                sb1.tile([P, 2, CT, VW], FP8, tag=f"vaug{n}", name=f"vaug{n}")
                for n in range(2)
            ]
            khat_bufs = [
                sb1.tile([P, CT, P, 2], FP8, tag=f"khat{n}", name=f"khat{n}")
                for n in range(2)
            ]
            for n in range(2):
                nc.sync.dma_start(
                    v_aug_bufs[n][:, :, :, 128:VW],
                    onescol_d[:].rearrange(
                        "p (b t c) -> p b t c", b=2, t=CT
                    ),
                )

            def emit_sacc(pr):
                kh = khat_bufs[pr % 2]
                va = v_aug_bufs[pr % 2]
                for t in range(CT):
                    nc.tensor.matmul(
                        s_tiles[t][:],
                        kh[:, t, :, :],
                        va[:, :, t, :],
                        start=(pr == 0),
                        stop=(pr == NPAIR - 1),
                        perf_mode=DRS,
                    )

            for i in range(NT):
                # paired input DMAs, issued six chunks ahead (pairs 0-2 were
                # issued before the loop)
                if i % 2 == 0 and i + 6 < NT:
                    nc.sync.dma_start(
                        xx_all[:, i + 6 : i + 8], xx_v[:, i + 6 : i + 8]
                    )
                # pass-2/epilogue consts: spread one per chunk so they never
                # clump the SP trigger queue against the xx prefetches
                if i == 3:
                    nc.sync.dma_start(
                        wqb[:],
                        wqb_d[:].rearrange(
                            "p (a s j b) -> p a s j b", a=TP, s=CT, b=2
                        ),
                    )
                elif i == 5:
                    nc.sync.dma_start(
                        wr[:], wr_d[:].rearrange("p (t j) -> p t j", t=CT)
                    )
                elif i == 7:
                    nc.sync.dma_start(
                        brb[:], brb_d[:].rearrange("p (t j) -> p t j", t=CT)
                    )
                elif i == 9:
                    nc.sync.dma_start(ebq[:], ebq_d[:])
                    nc.sync.dma_start(bvb[:], bvb_d[:])
                elif i == 11:
                    nc.sync.dma_start(
                        obd[:], obd_d[:].rearrange("p (s q) -> p s q", s=CT)
                    )
                elif i == 13:
                    nc.sync.dma_start(id16[:], id16_d[:])
                elif i >= 17 and i % 2 == 1:
                    g_pre = (i - 17) // 2   # prefetch y for pass 2
                    nc.sync.dma_start(yTg_all[:, g_pre], yTg_v[:, g_pre])

                vpre = ps_v.tile([P, Ch], F32, tag="vpre")
                for tp in range(TP):
                    nc.tensor.matmul(
                        vpre[:],
                        xx_all[:, i, 0, tp, :, :],
                        wv[:, tp, :, :],
                        start=(tp == 0),
                        stop=(tp == TP - 1),
                        perf_mode=DRS,
                    )
                # previous pair's S accumulation, delayed one chunk so its
                # khat LDWEIGHTS pulls ahead under the vpre stream instead
                # of chasing the Act exp
                if i >= 2 and i % 2 == 0:
                    emit_sacc(i // 2 - 1)
                v_aug = v_aug_bufs[(i // 2) % 2]
                nc.vector.tensor_copy(
                    v_aug[:, i % 2, :, 0:128],
                    vpre[:].rearrange("p (t q) -> p t q", t=CT),
                )

                kpre = ps_k.tile([P, Ch], F32, tag="kpre")
                for tp in range(TP):
                    nc.tensor.matmul(
                        kpre[:],
                        xx_all[:, i, 1, tp, :, :],
                        wk[:, tp, :, :],
                        start=(tp == 0),
                        stop=(tp == TP - 1),
                        perf_mode=DRS,
                    )
                khat8 = khat_bufs[(i // 2) % 2]
                if i == NT - 1:
                    # split the final exp per block so the last s_acc
                    # matmuls (and the chain behind them) start a
                    # quarter-exp earlier instead of idling the PE
                    for t in range(CT):
                        nc.scalar.activation(
                            khat8[:, t, :, i % 2],
                            kpre[:, P * t : P * (t + 1)],
                            AF.Exp,
                            scale=1.0 / W_SCALE,
                        )
                else:
                    nc.scalar.activation(
                        khat8[:, :, :, i % 2],
                        kpre[:].rearrange("p (t q) -> p t q", t=CT),
                        AF.Exp,
                        scale=1.0 / W_SCALE,
                    )
            emit_sacc(NPAIR - 1)

            # ------- epilogue: ctx = S * zkinv * e^{bq}  (x16 scale) --------
            for t in range(CT):
                nc.vector.reciprocal(
                    zkinv[:, t : t + 1], s_tiles[t][:, 128:129]
                )
            # fold the query-bias factor into the ctx rows (kch partitions)
            nc.vector.tensor_mul(zkinv[:], zkinv[:], ebq[:])
            for t in range(CT):
                nc.vector.tensor_copy(ctxR[:, t, :], bvb[:, P * t : P * (t + 1)])
                for blk in range(2):
                    p0 = 64 * blk
                    nc.vector.scalar_tensor_tensor(
                        ctxR[p0 : p0 + 64, t, p0 : p0 + 64],
                        s_tiles[t][p0 : p0 + 64, p0 : p0 + 64],
                        zkinv[p0 : p0 + 64, t : t + 1],
                        bvb[p0 : p0 + 64, P * t + p0 : P * t + p0 + 64],
                        op0=mybir.AluOpType.mult,
                        op1=mybir.AluOpType.add,
                    )

        # ---------------- pass 2: q softmax, attend, reproject ---------------
        # s-pair granularity; the reprojection runs one group late so the PE
        # never stalls waiting for the DVE/Pool multiply chain.
        with (
            tc.tile_pool(name="io2", bufs=2) as io2,
            tc.tile_pool(name="sb2", bufs=2) as sb2,
            tc.tile_pool(name="ps_q", bufs=2, space="PSUM") as ps_q,
            tc.tile_pool(name="ps_zb", bufs=1, space="PSUM") as ps_zb,
            tc.tile_pool(name="ps_o", bufs=2, space="PSUM") as ps_o,
        ):
            qn_bufs = [
                sb2.tile([P, TP, 2, GRP * P], FP8, tag=f"qn{n}", name=f"qn{n}")
                for n in range(2)
            ]

            def emit_opre_half(g, jp, dve_osb=False):
                qn = qn_bufs[g % 2]
                o_sb = io2.tile([P, 2, Ch], BF16, tag="osb")
                for jj in range(2):
                    j = 2 * jp + jj
                    opre = ps_o.tile([P, Ch], F32, tag="opre")
                    for sp in range(TP):
                        nc.tensor.matmul(
                            opre[:],
                            qn[:, sp, :, P * j : P * (j + 1)],
                            cw[:, sp, :, :],
                            start=(sp == 0),
                            stop=(sp == TP - 1),
                            perf_mode=DR,
                        )
                    # the tail groups run after all DVE softmax work is done,
                    # so splitting the copies Act/DVE halves the drain time
                    if dve_osb and jj == 1:
                        nc.vector.tensor_copy(o_sb[:, jj, :], opre[:])
                    else:
                        nc.scalar.copy(o_sb[:, jj, :], opre[:])
                i0 = g * GRP + 2 * jp
                nc.sync.dma_start(
                    out_d[P * i0 : P * (i0 + 2), :].rearrange(
                        "(u p) c -> p u c", u=2
                    ),
                    o_sb[:],
                )

            for g in range(NG):
                qhatT = sb2.tile([P, CT, GRP * P], BF16, tag="qhatT")
                zbinv = sb2.tile([P, CT, GRP * P], F32, tag="zbinv")
                qn = qn_bufs[g % 2]

                def emit_zb(sp, qhatT=qhatT, zbinv=zbinv, qn=qn):
                    zbp = ps_zb.tile([P, 2, GRP * P], F32, tag="zb")
                    for u in range(2):
                        nc.tensor.matmul(
                            zbp[:, u, :],
                            obd[:, 2 * sp + u, :],
                            qhatT[:, 2 * sp + u, :],
                            start=True,
                            stop=True,
                        )
                    nc.vector.reciprocal_approx_fast(
                        zbinv[:, 2 * sp : 2 * sp + 2, :], zbp[:]
                    )
                    # qn = qhat * (64/Z), one DVE op per s-pair
                    nc.vector.tensor_mul(
                        qn[:, sp, :, :],
                        qhatT[:, 2 * sp : 2 * sp + 2, :],
                        zbinv[:, 2 * sp : 2 * sp + 2, :],
                    )

                for sp in range(TP):
                    qpre = ps_q.tile([P, 2, GRP * P], F32, tag="qpre")
                    for u in range(2):
                        s = 2 * sp + u
                        for tp in range(TP):
                            nc.tensor.matmul(
                                qpre[:, u, :],
                                wqb[:, tp, s, :, :],
                                yTg_all[:, g, tp, :, :],
                                start=(tp == 0),
                                stop=(tp == TP - 1),
                                perf_mode=DRS,
                            )
                    nc.scalar.activation(
                        qhatT[:, 2 * sp : 2 * sp + 2, :],
                        qpre[:],
                        AF.Exp,
                        scale=1.0 / W_SCALE,
                    )
                    # zb[p,tok] = (1/64)*sum_{k in head(p)} e^{bq[k]} qhat[k]:
                    # bias-corrected Z/64 pre-broadcast on all 128 partitions.
                    # Delayed one pair so it never chases the Act exp; feeds
                    # opre one group later, so the reciprocal never gates PE.
                    if sp > 0:
                        emit_zb(sp - 1)

                if g == 0:
                    # CW[k,c] = e^{bq[k]}*(sum_v ctx[k,v] Wr[v,c] + 2 br_eff):
                    # ctx already carries e^{bq} (zkinv fold), brb_t rows are
                    # host-scaled. sum_k qn'[k] e^{bq[k]} per head = 64, so
                    # out_raw = 1024*(attended@Wr + br_eff). Emitted here so
                    # the wait on the ctx epilogue overlaps group 0's qpres.
                    for t in range(CT):
                        ctxT_ps = ps_q.tile([P, P], BF16, tag="qpre")
                        nc.tensor.transpose(ctxT_ps[:], ctxR[:, t, :], id16[:])
                        ctxT = sb2.tile([P, P], BF16, tag="ctxTs")
                        nc.scalar.copy(ctxT[:], ctxT_ps[:])
                        cw_ps = ps_q.tile([P, Ch], F32, tag="qpre")
                        nc.tensor.matmul(
                            cw_ps[:], ctxT[:], wr[:, t, :], start=True, stop=True
                        )
                        nc.vector.tensor_add(
                            cw[:, t // 2, t % 2, :], cw_ps[:], brb[:, t, :]
                        )
                    emit_zb(TP - 1)
                else:
                    # interleave the last zb between the two reprojection
                    # halves so its moving qhat never chases the Act exp and
                    # the fused divide starts early enough for the next group
                    emit_opre_half(g - 1, 0)
                    emit_zb(TP - 1)
                    emit_opre_half(g - 1, 1)

            emit_opre_half(NG - 1, 0, dve_osb=True)
            emit_opre_half(NG - 1, 1, dve_osb=True)

    nc.finalize()
    return nc


def _host_consts(Wk, bk, Wq, bq, Wv, bv, Wr, br):
    def rearr8(w):
        return (
            np.ascontiguousarray(
                (w * W_SCALE)
                .reshape(CT, P, Ch)
                .transpose(1, 0, 2)
                .reshape(P, CT * Ch)
            ).astype(FP8_NP)
        )

    # Wq x16, SwInterleave stationary layout [p, tp, s, j, b] with the
    # column reversal baked in so the HW reversal restores kch order:
    # wqb[p, tp, s, j, b] = 16*Wq[(2tp+b)*128 + p, s*128 + (127-j)]
    wq = (Wq * W_SCALE).reshape(TP, 2, P, CT, P)   # [tp, b, p, s, q]
    wq = wq[..., ::-1]                             # q -> j
    wqb = np.ascontiguousarray(
        wq.transpose(2, 0, 3, 4, 1).reshape(P, CT * CT * P)
    ).astype(FP8_NP)

    wr_r = np.ascontiguousarray(
        Wr.reshape(CT, P, Ch).transpose(1, 0, 2).reshape(P, CT * Ch)
    ).astype(BF16_NP)

    # e^{bq} rounded to bf16 once so numerator (cw rows) and denominator
    # (obd) use the *same* values; /64 below is exact (exponent shift).
    ebq = np.exp(bq.astype(np.float64)).astype(BF16_NP).astype(np.float32)
    ebq_col = np.ascontiguousarray(ebq.reshape(CT, P).T).astype(np.float32)

    # ctx is built WITHOUT bv (bv @ Wr is absorbed into br below, exact
    # because the query softmax weights sum to 1); bvb is all zeros so the
    # epilogue's fused multiply-add writes S*zkinv*e^{bq} on the diagonal.
    bvb = np.zeros((P, Ch), np.float32)
    br_eff = (
        br.astype(np.float64) + bv.astype(np.float64) @ Wr.astype(np.float64)
    ).astype(np.float32)

    # brb_t[p, t, c] = e^{bq[t*128+p]} * (W_SCALE/8) * br_eff[c]
    brb_t = (
        ebq.reshape(CT, P).transpose(1, 0)[:, :, None]
        * (br_eff[None, None, :] * (W_SCALE / 8.0))
    ).reshape(P, CT * Ch)

    # per-s blockdiag stationaries: obd[p, s, m] = e^{bq[s*128+p]}/64 where
    # p, m are in the same head-half (the e^{bq} factor rides the contraction
    # row, i.e. the absolute kch index k = s*128 + p)
    mask = np.zeros((P, P), np.float32)
    mask[0:64, 0:64] = 1.0
    mask[64:128, 64:128] = 1.0
    ebq_sp = ebq.reshape(CT, P)  # [s, p]
    obd = (
        mask[:, None, :] * ebq_sp.transpose(1, 0)[:, :, None] / QN_SCALE
    ).reshape(P, CT * P)
    # Wk's kch output columns are reversed within each 128-tile: the exp then
    # writes khat in SwInterleave physical order with plain strides, and the
    # DoubleRowSwInterleave hardware reversal restores kch order in s_acc.
    Wk_rev = np.ascontiguousarray(
        Wk.reshape(Ch, CT, P)[:, :, ::-1].reshape(Ch, Ch)
    )
    return {
        "Wk_r": rearr8(Wk_rev),
        "Wv_r": rearr8(Wv),
        "Wr_r": wr_r,
        "Wq_b": wqb,
        "brb_t": np.ascontiguousarray(brb_t).astype(np.float32),
        "ebq_col": ebq_col,
        "bv_blk": bvb.astype(BF16_NP),
        "onesbd": np.ascontiguousarray(obd).astype(BF16_NP),
        "ident16": np.eye(P).astype(BF16_NP),
        "ones_col": np.ones((P, 2 * CT * 8), FP8_NP),
    }


def _pack_xx(x, y):
    """SwInterleave stationary layout:
    xxsT[p, i, c, tp, j, b] = {x, x+y}[i*128 + (127-j), (2*tp+b)*128 + p].
    (A/B pairs interleaved along columns, column order reversed, per the
    DoubleRowSwInterleave hardware weight-stream convention.)"""

    def lay(a):
        t = a.reshape(NT, P, TP, 2, P).transpose(4, 0, 2, 3, 1)  # p i tp b q
        t = t[..., ::-1]                                         # q -> j
        return t.transpose(0, 1, 2, 4, 3)                        # p i tp j b

    return (
        np.stack([lay(x), lay(x + y)], axis=2)
        .astype(FP8_NP)
        .reshape(P, NT * 2 * CT * P)
    )


def _pack_y(y):
    """yTg[p, g, tp, u, j] = y[g*512+j, (2*tp+u)*128+p]."""
    return np.ascontiguousarray(
        y.reshape(NG, GRP * P, TP, 2, P)
        .transpose(4, 0, 2, 3, 1)
        .reshape(P, NG * CT * GRP * P)
    ).astype(FP8_NP)


_NC_CACHE = {}


def _get_nc():
    if "nc" not in _NC_CACHE:
        _NC_CACHE["nc"] = build_nc()
    return _NC_CACHE["nc"]


def kernel(input_, y, Wk, bk, Wq, bq, Wv, bv, Wr, br, _trace=False, _tmpdir=None):
    input_ = np.asarray(input_, np.float32)
    y = np.asarray(y, np.float32)
    consts = _host_consts(
        np.asarray(Wk, np.float32), np.asarray(bk, np.float32),
        np.asarray(Wq, np.float32), np.asarray(bq, np.float32),
        np.asarray(Wv, np.float32), np.asarray(bv, np.float32),
        np.asarray(Wr, np.float32), np.asarray(br, np.float32),
    )
    nc = _get_nc()
    in_maps = [
        {
            "xxsT": _pack_xx(input_[i], y[i]),
            "yTg": _pack_y(y[i]),
            **consts,
        }
        for i in range(B)
    ]
    res = run_bass_kernel_spmd(
        nc, in_maps, core_ids=list(range(B)), trace=_trace, tmpdir=_tmpdir
    )
    out = np.stack(
        [
            res.results[i]["out"].astype(np.float32) * (1.0 / OUT_SCALE)
            for i in range(B)
        ],
        axis=0,
    )
    if _trace:
        return out, res
    return out
